# revision 27
# baseline (speedup 1.0000x reference)
"""Barycentric-coordinates KNN kernel for Trainium2 (8 NeuronCores).

Pipeline (per core = one (batch, half-of-V) pair; 8 cores cover 4 batches x 2 halves):
  Phase 1 (device): negated squared distances via TensorE matmul rows
    [-2q,1]x[p,|p|^2] fused with ACT bias/negate; per-64-column-chunk top-8
    values+indices via DVE max8/max_index -> 512 candidates per query row.
  Host: exact top-33 merge (value desc, index asc), neighbor-coordinate
    gather, SHOT weight normalization (no per-partition gather exists on-chip).
  Phase 2 (device): weighted 3x3 covariance (fused multiply-accumulate),
    closed-form eigensolver (Newton on the characteristic cubic + cross
    products), SHOT sign disambiguation, tangent-plane log map, template-cell
    nearest-3 selection via bit-packed keys (dist^2 mantissa | k-slot) and
    max8, onehot payload extraction, barycentric weights.
  Host: decode k-slots from packed keys, pidx = nbr_idx[closest], assemble
    (4, 4096, 5, 8, 3, 2) output.
"""
import sys

sys.path.insert(0, "/opt/trn_rl_repo")

import numpy as np
from contextlib import ExitStack

import concourse.bass as bass
import concourse.mybir as mybir
import concourse.tile as tile
from concourse.bass_utils import run_bass_kernel_spmd
from concourse.tile import ScopedClock

f32 = np.float32
AF = mybir.ActivationFunctionType
ALU = mybir.AluOpType
DT = mybir.dt

B, V, K = 4, 4096, 32
HALF = V // 2            # queries per core
NT = HALF // 128         # 16 v-tiles per core
NCHUNK = 32              # phase-1 chunk count (chunk width 128)
CHUNKW = V // NCHUNK     # 128
CAND = NCHUNK * 8        # 256 candidates per row
R, A = 5, 8
NCELL = R * A            # 40 template cells
EPS = 1e-8
N_RADIAL, N_ANGULAR = 5, 8
TEMPLATE_RADIUS = 0.09

# ---------------------------------------------------------------------------
# Tile-framework workaround: walrus rejects instructions carrying more than a
# couple of sync waits. Spread extras across single-wait NOPs.
# ---------------------------------------------------------------------------


def _patched_drain_and_barrier(self, tick_clock, wait_clock):
    probe = self.nc.sync.nop(nofuse=True)
    wait_clock.add_sem_waits(probe.ins, ScopedClock({None: tick_clock.global_clock}))
    sync_info = probe.ins.sync_info
    waits = list(sync_info.on_wait or []) if sync_info is not None else []
    if len(waits) > 1:
        sync_info.on_wait = waits[:1]
        for i in range(1, len(waits)):
            extra = self.nc.sync.nop(nofuse=True)
            if extra.ins.sync_info is None:
                extra.ins.sync_info = mybir.SyncInfo(on_wait=[waits[i]], on_update=[])
            else:
                extra.ins.sync_info.on_wait = [waits[i]]
    self.nc.sync.drain()
    self.nc.all_engine_barrier()
    assert self.sems is not None
    popped = self.nc._tile_sem_poison_stack.pop()
    assert popped is self._sem_poison
    self.nc.clear_and_free_semaphores(list(self.sems.allocated().values()))
    self.nc.all_engine_barrier()


tile.TileContext._drain_and_barrier = _patched_drain_and_barrier


def split_sync_waits(nc, max_waits=1):
    for f in nc.m.functions:
        for b in f.blocks:
            new_list = []
            dirty = False
            for ins in b.instructions:
                si = ins.sync_info
                waits = list(si.on_wait) if (si is not None and si.on_wait) else []
                if len(waits) > max_waits:
                    dirty = True
                    extras, keep = waits[:-max_waits], waits[-max_waits:]
                    for j in range(0, len(extras), max_waits):
                        nop = mybir.InstNoOp(
                            name=f"I-wsplit-{nc.next_id()}", engine=ins.engine
                        )
                        nop.sync_info = mybir.SyncInfo(
                            on_wait=extras[j : j + max_waits], on_update=[]
                        )
                        new_list.append(nop)
                    si.on_wait = keep
                new_list.append(ins)
            if dirty:
                b.instructions = new_list


# ---------------------------------------------------------------------------
# Phase 1 program
# ---------------------------------------------------------------------------


def build_phase1():
    # -d2(q, p) via one 13-row fp16 split-precision matmul:
    #   2 q.p  = sum_c (ah_c + al_c)(bh_c + bl_c)  (al.bl term dropped)
    #   -|p|^2 = sph + spl,  -|q|^2 = sqh + sql    (hi/lo fp16 splits)
    # accumulated exactly in fp32 PSUM -> -d^2 with ~1e-6 abs error.
    # A 7-bit chunk-local index is packed into the mantissa low bits so a
    # single MAX8 per 128-wide chunk yields (value, index) fused; the host
    # decodes idx = bits & 127.
    nc = bass.Bass()
    NROW = 16
    pt5 = nc.declare_dram_parameter("pt5", [NROW, V], DT.float16, isOutput=False)
    qt5 = nc.declare_dram_parameter("qt5", [NROW, HALF], DT.float16, isOutput=False)
    candv_o = nc.declare_dram_parameter("candv", [HALF, CAND], DT.float32, isOutput=True)

    HC = NCHUNK // 2  # chunks per half (16)

    with tile.TileContext(nc) as tc, ExitStack() as ctx:
        cpool = ctx.enter_context(tc.tile_pool(name="const", bufs=1))
        npool = ctx.enter_context(tc.tile_pool(name="nkey", bufs=3))
        opool = ctx.enter_context(tc.tile_pool(name="cand", bufs=4))
        ppool = ctx.enter_context(tc.tile_pool(name="psum", bufs=2, space="PSUM"))

        pt = cpool.tile([NROW, V], DT.float16)
        qt = cpool.tile([NROW, HALF], DT.float16)
        J7 = cpool.tile([128, 2048], DT.int32)
        M128 = cpool.tile([128, 1], DT.int32)
        nc.sync.dma_start(pt[:], pt5[:])
        nc.sync.dma_start(qt[:], qt5[:])
        nc.gpsimd.iota(J7[:], pattern=[[0, HC], [1, CHUNKW]], base=0,
                       channel_multiplier=0)
        nc.vector.memset(M128[:], -128)

        for t in range(NT):
            for jh in range(2):
                ps = ppool.tile([128, 2048], DT.float32, space="PSUM")
                for k4 in range(4):
                    nc.tensor.matmul(
                        ps[:, k4 * 512:(k4 + 1) * 512],
                        qt[:, t * 128:(t + 1) * 128],
                        pt[:, jh * 2048 + k4 * 512: jh * 2048 + (k4 + 1) * 512],
                        start=True, stop=True,
                    )
                nk = npool.tile([128, 2048], DT.float32, tag="nk")
                nc.vector.scalar_tensor_tensor(
                    out=nk[:].bitcast(DT.int32), in0=ps[:].bitcast(DT.int32),
                    scalar=M128[:], in1=J7[:], op0=ALU.bitwise_and,
                    op1=ALU.bitwise_or)
                cv = opool.tile([128, HC * 8], DT.float32, tag="cv")
                for c in range(HC):
                    nc.vector.max(out=cv[:, c * 8:(c + 1) * 8],
                                  in_=nk[:, c * CHUNKW:(c + 1) * CHUNKW])
                nc.sync.dma_start(
                    candv_o[t * 128:(t + 1) * 128, jh * HC * 8:(jh + 1) * HC * 8],
                    cv[:])

    split_sync_waits(nc)
    return nc


# ---------------------------------------------------------------------------
# Phase 2 program
# ---------------------------------------------------------------------------


def _register_consts(nc, values):
    for value in values:
        t = nc.alloc_sbuf_tensor(f"const-float32-{value}", [128, 1], DT.float32)
        nc.gpsimd.memset(t.ap(), value)
        nc.const_aps.aps[(DT.float32, value)] = t.ap()
    nc.all_engine_barrier()


def build_phase2():
    nc = bass.Bass()
    _register_consts(nc, [0.5])
    ngh_i = nc.declare_dram_parameter("ngh", [HALF, 96], DT.float32, isOutput=False)
    wn_i = nc.declare_dram_parameter("wn", [HALF, K], DT.float32, isOutput=False)
    dd_i = nc.declare_dram_parameter("dd", [HALF, K], DT.float32, isOutput=False)
    # cst layout: [0:8]=-2cos(a), [8:16]=-2sin(a), [16:21]=r, [21:26]=r^2,
    #             [26:66]=tx, [66:106]=ty   (replicated over partitions)
    cst_i = nc.declare_dram_parameter("cst", [128, 106], DT.float32, isOutput=False)
    m3_o = nc.declare_dram_parameter("m3o", [HALF, NCELL, 3], DT.float32, isOutput=True)
    pxy_o = nc.declare_dram_parameter("pxy", [HALF, 2, K], DT.float32, isOutput=True)

    with tile.TileContext(nc) as tc, ExitStack() as ctx:
        cp = ctx.enter_context(tc.tile_pool(name="const", bufs=1))
        sp = ctx.enter_context(tc.tile_pool(name="scratch", bufs=2))
        bp = ctx.enter_context(tc.tile_pool(name="bc", bufs=2))

        NGH = cp.tile([128, NT, 96], DT.float32)
        WN = cp.tile([128, NT, K], DT.float32)
        DD = cp.tile([128, NT, K], DT.float32)
        CST = cp.tile([128, 106], DT.float32)
        nc.sync.dma_start(NGH[:], ngh_i[:].rearrange("(t p) c -> p t c", p=128))
        nc.sync.dma_start(WN[:], wn_i[:].rearrange("(t p) c -> p t c", p=128))
        nc.sync.dma_start(DD[:], dd_i[:].rearrange("(t p) c -> p t c", p=128))
        nc.sync.dma_start(CST[:], cst_i[:])
        TX = CST[:, 26:66]
        TY = CST[:, 66:106]

        KIOTA = cp.tile([128, NCELL, K], DT.int32)
        nc.gpsimd.iota(KIOTA[:], pattern=[[0, NCELL], [1, K]], base=-2147483648,
                       channel_multiplier=0)
        M32 = cp.tile([128, 1], DT.int32)
        nc.vector.memset(M32[:], -32)

        _tagn = [0]

        def nt_tile(pool=cp):
            _tagn[0] += 1
            return pool.tile([128, NT], DT.float32, tag=f"nt{_tagn[0]}",
                             name=f"nt{_tagn[0]}")

        NGH4 = NGH[:].rearrange("p t (c k) -> p t c k", c=3)

        # ---- covariance accumulation (batched over tiles) ----
        NW = cp.tile([128, NT, 96], DT.float32)
        NW4 = NW[:].rearrange("p t (c k) -> p t c k", c=3)
        nc.vector.tensor_tensor(
            out=NW4, in0=NGH4,
            in1=WN[:].rearrange("p t k -> p t () k").to_broadcast([128, NT, 3, K]),
            op=ALU.mult)
        CXX, CXY, CXZ, CYY, CYZ, CZZ = [nt_tile() for _ in range(6)]
        cov_dsts = {"xx": CXX, "xy": CXY, "xz": CXZ, "yy": CYY, "yz": CYZ, "zz": CZZ}
        pairs = [("xx", 0, 0), ("xy", 0, 1), ("xz", 0, 2),
                 ("yy", 1, 1), ("yz", 1, 2), ("zz", 2, 2)]
        for nmq, a, b in pairs:
            cj = sp.tile([128, NT, K], DT.float32, tag="covjunk")
            nc.gpsimd.tensor_tensor(out=cj[:], in0=NGH4[:, :, a, :],
                                    in1=NW4[:, :, b, :], op=ALU.mult)
            nc.vector.tensor_reduce(out=cov_dsts[nmq][:], in_=cj[:],
                                    axis=mybir.AxisListType.X, op=ALU.add)

        # ---- eigensolver on (128, NT) ----
        def tt(dst, a, bb, op):
            nc.vector.tensor_tensor(out=dst[:], in0=a[:], in1=bb[:], op=op)

        def sq_act(dst, a):
            nc.scalar.activation(dst[:], a[:], AF.Square)

        Q = nt_tile()
        tt(Q, CXX, CYY, ALU.add)
        tt(Q, Q, CZZ, ALU.add)
        nc.vector.tensor_scalar_mul(Q[:], Q[:], 1.0 / 3.0)
        BXX, BYY, BZZ = nt_tile(), nt_tile(), nt_tile()
        tt(BXX, CXX, Q, ALU.subtract)
        tt(BYY, CYY, Q, ALU.subtract)
        tt(BZZ, CZZ, Q, ALU.subtract)
        P2 = nt_tile()
        T1 = nt_tile(sp)
        sq_act(P2, BXX)
        sq_act(T1, BYY)
        tt(P2, P2, T1, ALU.add)
        sq_act(T1, BZZ)
        tt(P2, P2, T1, ALU.add)
        T2 = nt_tile(sp)
        sq_act(T1, CXY)
        sq_act(T2, CXZ)
        tt(T1, T1, T2, ALU.add)
        sq_act(T2, CYZ)
        tt(T1, T1, T2, ALU.add)
        nc.vector.tensor_scalar_mul(T1[:], T1[:], 2.0)
        tt(P2, P2, T1, ALU.add)
        PP = nt_tile()
        PPX = nt_tile()
        nc.vector.tensor_scalar_mul(PPX[:], P2[:], 1.0 / 6.0)

        def polished_sqrt(dst, x, tmp):
            # ACT Sqrt is ~7e-6; one Newton step s' = (s + x/s)/2 fixes it
            nc.scalar.activation(dst[:], x[:], AF.Sqrt)
            nc.vector.tensor_scalar_max(tmp[:], dst[:], 1e-30)
            nc.vector.reciprocal(tmp[:], tmp[:])
            nc.vector.tensor_tensor(out=tmp[:], in0=x[:], in1=tmp[:], op=ALU.mult)
            nc.vector.tensor_tensor(out=dst[:], in0=dst[:], in1=tmp[:], op=ALU.add)
            nc.vector.tensor_scalar_mul(dst[:], dst[:], 0.5)

        polished_sqrt(PP, PPX, T2)
        PINV = nt_tile()
        nc.vector.tensor_scalar_max(PINV[:], PP[:], 1e-20)
        nc.vector.reciprocal(PINV[:], PINV[:])
        NBXX, NBYY, NBZZ, NBXY, NBXZ, NBYZ = [nt_tile() for _ in range(6)]
        tt(NBXX, BXX, PINV, ALU.mult)
        tt(NBYY, BYY, PINV, ALU.mult)
        tt(NBZZ, BZZ, PINV, ALU.mult)
        tt(NBXY, CXY, PINV, ALU.mult)
        tt(NBXZ, CXZ, PINV, ALU.mult)
        tt(NBYZ, CYZ, PINV, ALU.mult)
        # det(B̂)
        DET = nt_tile()
        sq_act(T1, NBYZ)                     # byz^2
        tt(T2, NBYY, NBZZ, ALU.mult)
        tt(T2, T2, T1, ALU.subtract)
        tt(DET, NBXX, T2, ALU.mult)          # + bxx (byy bzz - byz^2)
        tt(T1, NBXY, NBZZ, ALU.mult)
        tt(T2, NBYZ, NBXZ, ALU.mult)
        tt(T1, T1, T2, ALU.subtract)
        tt(T1, NBXY, T1, ALU.mult)
        tt(DET, DET, T1, ALU.subtract)       # - bxy (bxy bzz - byz bxz)
        tt(T1, NBXY, NBYZ, ALU.mult)
        tt(T2, NBYY, NBXZ, ALU.mult)
        tt(T1, T1, T2, ALU.subtract)
        tt(T1, NBXZ, T1, ALU.mult)
        tt(DET, DET, T1, ALU.add)            # + bxz (bxy byz - byy bxz)
        R2 = nt_tile()                       # 2r = det  clamped to [-2, 2]
        nc.vector.tensor_scalar_min(R2[:], DET[:], 2.0)
        nc.vector.tensor_scalar_max(R2[:], R2[:], -2.0)

        def newton(beta0):
            BETA = nt_tile()
            nc.vector.memset(BETA[:], beta0)
            FV = nt_tile(sp)
            B2 = nt_tile(sp)
            for _ in range(8):
                sq_act(B2, BETA)                              # β²
                tt(FV, B2, BETA, ALU.mult)                    # β³
                nc.vector.scalar_tensor_tensor(
                    out=T1[:], in0=BETA[:], scalar=3.0, in1=FV[:],
                    op0=ALU.mult, op1=ALU.subtract)           # 3β - β³ ... careful sign
                # T1 = (β*3) - β³  => f = β³-3β-2r = -(T1) - 2r
                tt(T1, T1, R2, ALU.add)                       # T1 = 3β - β³ + 2r = -f
                nc.vector.tensor_scalar(out=B2[:], in0=B2[:], scalar1=3.0,
                                        scalar2=-3.0, op0=ALU.mult, op1=ALU.add)  # f' = 3β²-3
                nc.vector.tensor_scalar_max(B2[:], B2[:], 1e-8)
                nc.vector.reciprocal(B2[:], B2[:])
                tt(T1, T1, B2, ALU.mult)                      # -f/f'
                tt(BETA, BETA, T1, ALU.add)                   # β - f/f'
            return BETA

        BMAX = newton(2.2)
        BMIN = newton(-2.2)
        LMAX = nt_tile()
        LMIN = nt_tile()
        tt(LMAX, PP, BMAX, ALU.mult)
        tt(LMAX, LMAX, Q, ALU.add)
        tt(LMIN, PP, BMIN, ALU.mult)
        tt(LMIN, LMIN, Q, ALU.add)

        def evec(lam):
            # columns of A - lam I
            D0, D1, D2 = nt_tile(sp), nt_tile(sp), nt_tile(sp)
            tt(D0, CXX, lam, ALU.subtract)
            tt(D1, CYY, lam, ALU.subtract)
            tt(D2, CZZ, lam, ALU.subtract)
            m0 = (D0, CXY, CXZ)
            m1 = (CXY, D1, CYZ)
            m2 = (CXZ, CYZ, D2)

            def cross(u, v):
                rx, ry, rz = nt_tile(sp), nt_tile(sp), nt_tile(sp)
                tt(rx, u[1], v[2], ALU.mult)
                tt(T1, u[2], v[1], ALU.mult)
                tt(rx, rx, T1, ALU.subtract)
                tt(ry, u[2], v[0], ALU.mult)
                tt(T1, u[0], v[2], ALU.mult)
                tt(ry, ry, T1, ALU.subtract)
                tt(rz, u[0], v[1], ALU.mult)
                tt(T1, u[1], v[0], ALU.mult)
                tt(rz, rz, T1, ALU.subtract)
                return rx, ry, rz

            def norm2(c):
                n = nt_tile(sp)
                sq_act(n, c[0])
                sq_act(T1, c[1])
                tt(n, n, T1, ALU.add)
                sq_act(T1, c[2])
                tt(n, n, T1, ALU.add)
                return n

            c01 = cross(m0, m1)
            c02 = cross(m0, m2)
            c12 = cross(m1, m2)
            n01, n02, n12 = norm2(c01), norm2(c02), norm2(c12)
            G1, G2, G3 = nt_tile(sp), nt_tile(sp), nt_tile(sp)
            tt(G1, n01, n02, ALU.is_ge)
            tt(G2, n01, n12, ALU.is_ge)
            tt(G1, G1, G2, ALU.mult)                    # pick01
            tt(G3, n02, n12, ALU.is_ge)
            U = nt_tile(sp)
            nc.vector.tensor_scalar(out=U[:], in0=G1[:], scalar1=-1.0, scalar2=1.0,
                                    op0=ALU.mult, op1=ALU.add)   # 1 - pick01
            tt(G2, U, G3, ALU.mult)                     # pick02
            nc.vector.tensor_scalar(out=G3[:], in0=G3[:], scalar1=-1.0, scalar2=1.0,
                                    op0=ALU.mult, op1=ALU.add)   # 1 - g3
            tt(G3, U, G3, ALU.mult)                     # pick12
            out = []
            for ci in range(3):
                VC = nt_tile()
                tt(VC, c01[ci], G1, ALU.mult)
                tt(T1, c02[ci], G2, ALU.mult)
                tt(VC, VC, T1, ALU.add)
                tt(T1, c12[ci], G3, ALU.mult)
                tt(VC, VC, T1, ALU.add)
                out.append(VC)
            n2v = norm2(out)
            n = nt_tile(sp)
            polished_sqrt(n, n2v, T1)
            nc.vector.tensor_scalar_max(n[:], n[:], 1e-30)
            nc.vector.reciprocal(n[:], n[:])
            for VC in out:
                tt(VC, VC, n, ALU.mult)
            return out

        ZAX = evec(LMIN)
        XAX = evec(LMAX)

        # ---- disambiguation dots (batched over tiles) ----
        def batched_dot(AX, DST, engines=("vector", "gpsimd", "vector")):
            # DST = sum_c NGH[:, :, c, :] * AX[c] broadcast over K
            tmp = sp.tile([128, NT, K], DT.float32, tag="dottmp")
            axb = [AX[c][:].rearrange("p t -> p t ()").to_broadcast([128, NT, K])
                   for c in range(3)]
            nc.vector.tensor_tensor(out=DST[:], in0=NGH4[:, :, 0, :], in1=axb[0],
                                    op=ALU.mult)
            nc.gpsimd.tensor_tensor(out=tmp[:], in0=NGH4[:, :, 1, :], in1=axb[1],
                                    op=ALU.mult)
            nc.vector.tensor_tensor(out=DST[:], in0=DST[:], in1=tmp[:], op=ALU.add)
            nc.gpsimd.tensor_tensor(out=tmp[:], in0=NGH4[:, :, 2, :], in1=axb[2],
                                    op=ALU.mult)
            nc.vector.tensor_tensor(out=DST[:], in0=DST[:], in1=tmp[:], op=ALU.add)

        DOTX = cp.tile([128, NT, K], DT.float32)
        DOTZ = cp.tile([128, NT, K], DT.float32)
        batched_dot(XAX, DOTX)
        batched_dot(ZAX, DOTZ)

        SG = cp.tile([128, NT, K], DT.float32)
        FX = nt_tile()
        FZ = nt_tile()
        for DOT, F in ((DOTX, FX), (DOTZ, FZ)):
            nc.scalar.activation(SG[:], DOT[:], AF.Sign)
            nc.vector.tensor_reduce(out=F[:], in_=SG[:], axis=mybir.AxisListType.X,
                                    op=ALU.add)
            nc.scalar.activation(F[:], F[:], AF.Sign, bias=0.5, scale=1.0)
        for c in range(3):
            tt(XAX[c], XAX[c], FX, ALU.mult)
            tt(ZAX[c], ZAX[c], FZ, ALU.mult)
        nc.vector.tensor_tensor(
            out=DOTX[:], in0=DOTX[:],
            in1=FX[:].rearrange("p t -> p t ()").to_broadcast([128, NT, K]),
            op=ALU.mult)
        # y = cross(z, x)
        YAX = []
        for (i1, i2) in ((1, 2), (2, 0), (0, 1)):
            YC = nt_tile()
            tt(YC, ZAX[i1], XAX[i2], ALU.mult)
            tt(T1, ZAX[i2], XAX[i1], ALU.mult)
            tt(YC, YC, T1, ALU.subtract)
            YAX.append(YC)
        DOTY = cp.tile([128, NT, K], DT.float32)
        batched_dot(YAX, DOTY)

        # ---- projections (batched over all tiles) -> PXY (p, t, xy, k) ----
        PXY = cp.tile([128, NT, 2, K], DT.float32)
        PX = PXY[:][:, :, 0, :]
        PY = PXY[:][:, :, 1, :]
        SC = cp.tile([128, NT, K], DT.float32)
        U2 = cp.tile([128, NT, K], DT.float32)
        nc.scalar.activation(SC[:], DOTX[:], AF.Square)
        nc.scalar.activation(U2[:], DOTY[:], AF.Square)
        nc.vector.tensor_tensor(out=U2[:], in0=SC[:], in1=U2[:], op=ALU.add)
        nc.scalar.activation(SC[:], U2[:], AF.Sqrt)
        # one Newton step: s' = 0.5 (s + u/s) makes sqrt correctly-rounded-ish
        RCN = cp.tile([128, NT, K], DT.float32)
        nc.vector.tensor_scalar_max(RCN[:], SC[:], 1e-30)
        nc.vector.reciprocal(RCN[:], RCN[:])
        nc.vector.tensor_tensor(out=RCN[:], in0=U2[:], in1=RCN[:], op=ALU.mult)
        nc.vector.tensor_tensor(out=SC[:], in0=SC[:], in1=RCN[:], op=ALU.add)
        nc.vector.tensor_scalar(out=SC[:], in0=SC[:], scalar1=0.5, scalar2=EPS,
                                op0=ALU.mult, op1=ALU.add)
        nc.vector.reciprocal(SC[:], SC[:])
        nc.vector.tensor_tensor(out=SC[:], in0=SC[:], in1=DD[:], op=ALU.mult)
        nc.vector.tensor_tensor(out=PX, in0=DOTX[:], in1=SC[:], op=ALU.mult)
        nc.vector.tensor_tensor(out=PY, in0=DOTY[:], in1=SC[:], op=ALU.mult)
        # S2 = px^2 + py^2
        S2 = cp.tile([128, NT, K], DT.float32)
        S2T = cp.tile([128, NT, K], DT.float32)
        nc.scalar.activation(S2[:], PX, AF.Square)
        nc.scalar.activation(S2T[:], PY, AF.Square)
        nc.vector.tensor_tensor(out=S2[:], in0=S2[:], in1=S2T[:], op=ALU.add)
        nc.sync.dma_start(pxy_o[:].rearrange("(t p) x k -> p t x k", p=128), PXY[:])

        # ---- BC selection per tile ----
        # Key for cell (i,j), slot k:  d2 = (S2[k] + r_i^2) + r_i * W8[j,k]
        # with W8[j,k] = -2 (cos_j px[k] + sin_j py[k]); then pack slot bits
        # and take the top-3 keys per cell via MAX8. Winner coordinates are
        # gathered on the host from pxy_o (it gathers pidx anyway).
        COSB = CST[:, 0:8].rearrange("p a -> p a ()").to_broadcast([128, A, K])
        SINB = CST[:, 8:16].rearrange("p a -> p a ()").to_broadcast([128, A, K])
        RB = CST[:, 16:21].rearrange("p r -> p r () ()").to_broadcast([128, R, A, K])
        R2B = CST[:, 21:26].rearrange("p r -> p r ()").to_broadcast([128, R, K])
        for t in range(NT):
            pxb = PX[:, t, :].rearrange("p k -> p () k").to_broadcast([128, A, K])
            pyb = PY[:, t, :].rearrange("p k -> p () k").to_broadcast([128, A, K])
            T8 = bp.tile([128, A, K], DT.float32, tag="t8")
            W8 = bp.tile([128, A, K], DT.float32, tag="w8")
            nc.gpsimd.tensor_tensor(out=T8[:], in0=pxb, in1=COSB, op=ALU.mult)
            nc.gpsimd.tensor_tensor(out=W8[:], in0=pyb, in1=SINB, op=ALU.mult)
            nc.gpsimd.tensor_tensor(out=W8[:], in0=W8[:], in1=T8[:], op=ALU.add)
            S2R2 = bp.tile([128, R, K], DT.float32, tag="s2r2")
            nc.gpsimd.tensor_tensor(
                out=S2R2[:],
                in0=S2[:, t, :].rearrange("p k -> p () k").to_broadcast([128, R, K]),
                in1=R2B, op=ALU.add)
            RW = bp.tile([128, R, A, K], DT.float32, tag="rw")
            nc.gpsimd.tensor_tensor(
                out=RW[:], in0=RB,
                in1=W8[:].rearrange("p a k -> p () a k").to_broadcast([128, R, A, K]),
                op=ALU.mult)
            KEY = bp.tile([128, R, A, K], DT.float32, tag="key")
            nc.gpsimd.tensor_tensor(
                out=KEY[:], in0=RW[:],
                in1=S2R2[:].rearrange("p r k -> p r () k").to_broadcast([128, R, A, K]),
                op=ALU.add)
            NKEY = bp.tile([128, NCELL, K], DT.float32, tag="nkey", bufs=3)
            nc.vector.scalar_tensor_tensor(
                out=NKEY[:].bitcast(DT.int32),
                in0=KEY[:].rearrange("p r a k -> p (r a) k").bitcast(DT.int32),
                scalar=M32[:], in1=KIOTA[:], op0=ALU.bitwise_and,
                op1=ALU.bitwise_or)
            M8 = bp.tile([128, NCELL, 8], DT.float32, tag="m8", bufs=3)
            for ra in range(NCELL):
                nc.vector.max(out=M8[:, ra, :], in_=NKEY[:, ra, :])
            M3C = bp.tile([128, NCELL, 3], DT.float32, tag="m3c", bufs=3)
            nc.scalar.copy(M3C[:], M8[:, :, 0:3])
            nc.sync.dma_start(m3_o[t * 128:(t + 1) * 128, :, :], M3C[:])

    split_sync_waits(nc)
    return nc


# ---------------------------------------------------------------------------
# Host glue
# ---------------------------------------------------------------------------


def _split16(x):
    """f32 -> (hi, lo) fp16 pair with hi + lo ~= x."""
    hi = x.astype(np.float16)
    lo = (x - hi.astype(f32)).astype(np.float16)
    return hi, lo


def host_prep_phase1(vertices):
    """vertices (4, 4096, 3) -> list of 8 input maps (13-row fp16 split)."""
    f16 = np.float16
    maps = []
    for core in range(8):
        b, h = core // 2, core % 2
        verts = np.ascontiguousarray(vertices[b], dtype=f32)
        sq = (verts.astype(np.float64) ** 2).sum(-1).astype(f32)
        bh, bl = _split16(verts.T)                     # (3, V) each
        sph, spl = _split16(-sq[None, :])              # (1, V)
        onev = np.ones((1, V), f16)
        pt5 = np.concatenate(
            [bh, bh, bl, bl, sph, spl, onev, onev], axis=0).astype(f16)
        Q = verts[h * HALF:(h + 1) * HALF]
        qsq = sq[h * HALF:(h + 1) * HALF]
        ah, al = _split16(2.0 * Q.T.astype(f32))       # (3, HALF)
        sqh, sql = _split16(-qsq[None, :])
        oneq = np.ones((1, HALF), f16)
        qt5 = np.concatenate(
            [ah, al, ah, al, oneq, oneq, sqh, sql], axis=0).astype(f16)
        maps.append({"pt5": np.ascontiguousarray(pt5),
                     "qt5": np.ascontiguousarray(qt5)})
    return maps


def host_merge(candv, verts, Q):
    """Decode packed candidates, refine exactly, take top-33 by (d2, index).

    candv (HALF, CAND) f32: bits = (-d2 & ~127) | chunk_local_idx, column c
    belongs to chunk c // 8. The device d2 is approximate (fp16-split matmul
    + 7 masked mantissa bits); the top ~48 candidates are re-scored with
    exact f32 distances so the top-33 boundary is noise-free.
    -> nbr (HALF,32) int64, d (HALF,32), radius (HALF,).
    """
    NCAND = 48
    bits = candv.view(np.uint32)
    j = (bits & np.uint32(127)).astype(np.int64)
    d2m = -(bits & np.uint32(0xFFFFFF80)).view(f32)
    chunk = np.arange(CAND, dtype=np.int64) // 8
    gidx = chunk[None, :] * CHUNKW + j
    part = np.argpartition(d2m, NCAND, axis=1)[:, :NCAND]
    cd = np.take_along_axis(gidx, part, axis=1)              # (HALF, 48)
    diff = verts[cd] - Q[:, None, :]
    d2x = np.einsum("qkc,qkc->qk", diff, diff, dtype=f32).astype(f32)
    order = np.lexsort((cd, d2x), axis=1)[:, :33]
    vals = np.take_along_axis(d2x, order, axis=1)
    idxs = np.take_along_axis(cd, order, axis=1)
    d33 = np.sqrt(np.maximum(vals, 0.0)).astype(f32)
    return idxs[:, :32], d33[:, :32], d33[:, 32]


def host_prep_phase2(vertices, template, p1_results):
    """Build phase-2 input maps + per-core nbr tables from phase-1 outputs."""
    template = np.asarray(template, f32)
    tx = template[..., 0].reshape(-1).astype(f32)
    ty = template[..., 1].reshape(-1).astype(f32)
    # polar factorization of the template grid (it is a polar r x a grid)
    r64 = np.hypot(template[..., 0].astype(np.float64),
                   template[..., 1].astype(np.float64)).mean(axis=1)  # (R,)
    ang = np.arctan2(template[-1, :, 1].astype(np.float64),
                     template[-1, :, 0].astype(np.float64))           # (A,)
    cst_row = np.concatenate([
        -2.0 * np.cos(ang), -2.0 * np.sin(ang), r64, r64 * r64,
        tx.astype(np.float64), ty.astype(np.float64)]).astype(f32)
    cst = np.ascontiguousarray(np.broadcast_to(cst_row[None, :], (128, 106)))
    maps, nbrs = [], []
    for core in range(8):
        b, h = core // 2, core % 2
        verts = np.ascontiguousarray(vertices[b], dtype=f32)
        cv = p1_results[core]["candv"]
        Q = verts[h * HALF:(h + 1) * HALF]
        nbr, d, radius = host_merge(cv, verts, Q)
        neigh = (verts[nbr] - Q[:, None, :]).astype(f32)          # (HALF, 32, 3)
        ngh = np.ascontiguousarray(neigh.transpose(0, 2, 1).reshape(HALF, 96))
        w = (radius[:, None] - d).astype(f32)
        wn = (w / (w.sum(1, keepdims=True, dtype=f32) + f32(EPS))).astype(f32)
        maps.append({"ngh": ngh, "wn": np.ascontiguousarray(wn),
                     "dd": np.ascontiguousarray(d), "cst": cst})
        nbrs.append(nbr)
    return maps, nbrs


def host_assemble(p2_results, nbrs, template):
    """Decode slots, gather winner coords, barycentric weights, assemble output."""
    template = np.asarray(template, f32)
    txy = template.reshape(NCELL, 2)
    out = np.zeros((B, V, R, A, 3, 2), f32)
    one = f32(1.0)
    for core in range(8):
        b, h = core // 2, core % 2
        m3 = np.ascontiguousarray(p2_results[core]["m3o"])        # (HALF, 40, 3)
        pxy = p2_results[core]["pxy"]                             # (HALF, 2, 32)
        k3 = (m3.view(np.int32) & 31).astype(np.int64)            # (HALF, 40, 3)
        nbr = nbrs[core]                                          # (HALF, 32)
        pidx = np.take_along_axis(nbr[:, None, :].repeat(NCELL, 1), k3, axis=2)
        k3f = k3.reshape(HALF, NCELL * 3)
        px = np.take_along_axis(pxy[:, 0, :], k3f, axis=1).reshape(HALF, NCELL, 3)
        py = np.take_along_axis(pxy[:, 1, :], k3f, axis=1).reshape(HALF, NCELL, 3)
        p0x, p1x, p2x = px[..., 0], px[..., 1], px[..., 2]
        p0y, p1y, p2y = py[..., 0], py[..., 1], py[..., 2]
        v0x, v0y = p2x - p0x, p2y - p0y
        v1x, v1y = p1x - p0x, p1y - p0y
        v2x = txy[None, :, 0] - p0x
        v2y = txy[None, :, 1] - p0y
        d00 = v0x * v0x + v0y * v0y
        d01 = v0x * v1x + v0y * v1y
        d02 = v0x * v2x + v0y * v2y
        d11 = v1x * v1x + v1y * v1y
        d12 = v1x * v2x + v1y * v2y
        den = d00 * d11 - d01 * d01 + f32(1e-6)
        w2 = (d11 * d02 - d01 * d12) / den
        w1 = (d00 * d12 - d01 * d02) / den
        w0 = one - w2 - w1
        weights = np.stack([w2, w1, w0], axis=-1).astype(f32)     # (HALF, 40, 3)
        sl = slice(h * HALF, (h + 1) * HALF)
        out[b, sl, ..., 0] = pidx.reshape(HALF, R, A, 3).astype(f32)
        out[b, sl, ..., 1] = weights.reshape(HALF, R, A, 3)
    return out


_PROGS = {}


def _prog(name):
    if name not in _PROGS:
        _PROGS[name] = build_phase1() if name == "p1" else build_phase2()
    return _PROGS[name]


def run_phase1(vertices, trace=False):
    maps = host_prep_phase1(vertices)
    return run_bass_kernel_spmd(_prog("p1"), maps, list(range(8)), trace=trace)


def kernel(vertices, template, trace=False, _timing=None):
    vertices = np.asarray(vertices, f32)
    template = np.asarray(template, f32)
    r1 = run_bass_kernel_spmd(_prog("p1"), host_prep_phase1(vertices),
                              list(range(8)), trace=trace)
    maps2, nbrs = host_prep_phase2(vertices, template, r1.results)
    r2 = run_bass_kernel_spmd(_prog("p2"), maps2, list(range(8)), trace=trace)
    if _timing is not None:
        _timing["phase1"] = r1
        _timing["phase2"] = r2
        _timing["maps2"] = maps2
        _timing["nbrs"] = nbrs
    return host_assemble(r2.results, nbrs, template)


if __name__ == "__main__":
    # Phase-1 standalone check against exact numpy KNN.
    cache = np.load("/root/problem/dev_cache/ref.npz")
    vertices = cache["vertices"]
    res = run_phase1(vertices)
    nbad = 0
    for core in range(8):
        b, h = core // 2, core % 2
        verts = vertices[b].astype(f32)
        Q = verts[h * HALF:(h + 1) * HALF]
        d2 = ((Q[:, None, :] - verts[None, :, :]) ** 2).sum(-1)
        ref_order = np.argsort(d2, axis=1, kind="stable")[:, :33]
        nbr, d, rad = host_merge(res.results[core]["candv"], verts, Q)
        rnbr = ref_order[:, :32]
        idx_match = (np.sort(nbr, 1) == np.sort(rnbr, 1)).mean()
        rrad = np.sqrt(np.take_along_axis(d2, ref_order[:, 32:33], axis=1)[:, 0])
        print(f"core {core}: top32 set match={idx_match:.6f} "
              f"rad maxdiff={np.abs(rad-rrad).max():.2e}")
        nbad += (np.sort(nbr, 1) != np.sort(rnbr, 1)).sum()
    print("total nbr mismatches vs exact:", nbad)



# revision 30
# speedup vs baseline: 1.0939x; 1.0939x over previous
"""Barycentric-coordinates KNN kernel for Trainium2 (8 NeuronCores).

Pipeline (per core = one (batch, half-of-V) pair; 8 cores cover 4 batches x 2 halves):
  Phase 1 (device): negated squared distances via TensorE matmul rows
    [-2q,1]x[p,|p|^2] fused with ACT bias/negate; per-64-column-chunk top-8
    values+indices via DVE max8/max_index -> 512 candidates per query row.
  Host: exact top-33 merge (value desc, index asc), neighbor-coordinate
    gather, SHOT weight normalization (no per-partition gather exists on-chip).
  Phase 2 (device): weighted 3x3 covariance (fused multiply-accumulate),
    closed-form eigensolver (Newton on the characteristic cubic + cross
    products), SHOT sign disambiguation, tangent-plane log map, template-cell
    nearest-3 selection via bit-packed keys (dist^2 mantissa | k-slot) and
    max8, onehot payload extraction, barycentric weights.
  Host: decode k-slots from packed keys, pidx = nbr_idx[closest], assemble
    (4, 4096, 5, 8, 3, 2) output.
"""
import sys

sys.path.insert(0, "/opt/trn_rl_repo")

import numpy as np
from contextlib import ExitStack

import concourse.bass as bass
import concourse.mybir as mybir
import concourse.tile as tile
from concourse.bass_utils import run_bass_kernel_spmd
from concourse.tile import ScopedClock

f32 = np.float32
AF = mybir.ActivationFunctionType
ALU = mybir.AluOpType
DT = mybir.dt

B, V, K = 4, 4096, 32
HALF = V // 2            # queries per core
NT = HALF // 128         # 16 v-tiles per core
NCHUNK = 16              # phase-1 chunk count (chunk width 256)
CHUNKW = V // NCHUNK     # 128
CAND = NCHUNK * 8        # 256 candidates per row
R, A = 5, 8
NCELL = R * A            # 40 template cells
EPS = 1e-8
N_RADIAL, N_ANGULAR = 5, 8
TEMPLATE_RADIUS = 0.09

# ---------------------------------------------------------------------------
# Tile-framework workaround: walrus rejects instructions carrying more than a
# couple of sync waits. Spread extras across single-wait NOPs.
# ---------------------------------------------------------------------------


def _patched_drain_and_barrier(self, tick_clock, wait_clock):
    probe = self.nc.sync.nop(nofuse=True)
    wait_clock.add_sem_waits(probe.ins, ScopedClock({None: tick_clock.global_clock}))
    sync_info = probe.ins.sync_info
    waits = list(sync_info.on_wait or []) if sync_info is not None else []
    if len(waits) > 1:
        sync_info.on_wait = waits[:1]
        for i in range(1, len(waits)):
            extra = self.nc.sync.nop(nofuse=True)
            if extra.ins.sync_info is None:
                extra.ins.sync_info = mybir.SyncInfo(on_wait=[waits[i]], on_update=[])
            else:
                extra.ins.sync_info.on_wait = [waits[i]]
    self.nc.sync.drain()
    self.nc.all_engine_barrier()
    assert self.sems is not None
    popped = self.nc._tile_sem_poison_stack.pop()
    assert popped is self._sem_poison
    self.nc.clear_and_free_semaphores(list(self.sems.allocated().values()))
    self.nc.all_engine_barrier()


tile.TileContext._drain_and_barrier = _patched_drain_and_barrier


def split_sync_waits(nc, max_waits=1):
    for f in nc.m.functions:
        for b in f.blocks:
            new_list = []
            dirty = False
            for ins in b.instructions:
                si = ins.sync_info
                waits = list(si.on_wait) if (si is not None and si.on_wait) else []
                if len(waits) > max_waits:
                    dirty = True
                    extras, keep = waits[:-max_waits], waits[-max_waits:]
                    for j in range(0, len(extras), max_waits):
                        nop = mybir.InstNoOp(
                            name=f"I-wsplit-{nc.next_id()}", engine=ins.engine
                        )
                        nop.sync_info = mybir.SyncInfo(
                            on_wait=extras[j : j + max_waits], on_update=[]
                        )
                        new_list.append(nop)
                    si.on_wait = keep
                new_list.append(ins)
            if dirty:
                b.instructions = new_list


# ---------------------------------------------------------------------------
# Phase 1 program
# ---------------------------------------------------------------------------


def build_phase1():
    # -d2(q, p) via one 13-row fp16 split-precision matmul:
    #   2 q.p  = sum_c (ah_c + al_c)(bh_c + bl_c)  (al.bl term dropped)
    #   -|p|^2 = sph + spl,  -|q|^2 = sqh + sql    (hi/lo fp16 splits)
    # accumulated exactly in fp32 PSUM -> -d^2 with ~1e-6 abs error.
    # A 7-bit chunk-local index is packed into the mantissa low bits so a
    # single MAX8 per 128-wide chunk yields (value, index) fused; the host
    # decodes idx = bits & 127.
    nc = bass.Bass()
    NROW = 16
    pt5 = nc.declare_dram_parameter("pt5", [NROW, V], DT.float16, isOutput=False)
    qt5 = nc.declare_dram_parameter("qt5", [NROW, HALF], DT.float16, isOutput=False)
    candv_o = nc.declare_dram_parameter("candv", [HALF, CAND], DT.float32, isOutput=True)

    HC = NCHUNK // 2  # chunks per half (16)

    with tile.TileContext(nc) as tc, ExitStack() as ctx:
        cpool = ctx.enter_context(tc.tile_pool(name="const", bufs=1))
        npool = ctx.enter_context(tc.tile_pool(name="nkey", bufs=3))
        opool = ctx.enter_context(tc.tile_pool(name="cand", bufs=4))
        ppool = ctx.enter_context(tc.tile_pool(name="psum", bufs=2, space="PSUM"))

        pt = cpool.tile([NROW, V], DT.float16)
        qt = cpool.tile([NROW, HALF], DT.float16)
        J7 = cpool.tile([128, 2048], DT.int32)
        M128 = cpool.tile([128, 1], DT.int32)
        nc.sync.dma_start(pt[:], pt5[:])
        nc.sync.dma_start(qt[:], qt5[:])
        nc.gpsimd.iota(J7[:], pattern=[[0, HC], [1, CHUNKW]], base=0,
                       channel_multiplier=0)
        nc.vector.memset(M128[:], -256)

        for t in range(NT):
            for jh in range(2):
                ps = ppool.tile([128, 2048], DT.float32, space="PSUM")
                for k4 in range(4):
                    nc.tensor.matmul(
                        ps[:, k4 * 512:(k4 + 1) * 512],
                        qt[:, t * 128:(t + 1) * 128],
                        pt[:, jh * 2048 + k4 * 512: jh * 2048 + (k4 + 1) * 512],
                        start=True, stop=True,
                    )
                nk = npool.tile([128, 2048], DT.float32, tag="nk")
                nc.vector.scalar_tensor_tensor(
                    out=nk[:].bitcast(DT.int32), in0=ps[:].bitcast(DT.int32),
                    scalar=M128[:], in1=J7[:], op0=ALU.bitwise_and,
                    op1=ALU.bitwise_or)
                cv = opool.tile([128, HC * 8], DT.float32, tag="cv")
                for c in range(HC):
                    nc.vector.max(out=cv[:, c * 8:(c + 1) * 8],
                                  in_=nk[:, c * CHUNKW:(c + 1) * CHUNKW])
                nc.sync.dma_start(
                    candv_o[t * 128:(t + 1) * 128, jh * HC * 8:(jh + 1) * HC * 8],
                    cv[:])

    split_sync_waits(nc)
    return nc


# ---------------------------------------------------------------------------
# Phase 2 program
# ---------------------------------------------------------------------------


def _register_consts(nc, values):
    for value in values:
        t = nc.alloc_sbuf_tensor(f"const-float32-{value}", [128, 1], DT.float32)
        nc.gpsimd.memset(t.ap(), value)
        nc.const_aps.aps[(DT.float32, value)] = t.ap()
    nc.all_engine_barrier()


def build_phase2():
    nc = bass.Bass()
    _register_consts(nc, [0.5])
    ngh_i = nc.declare_dram_parameter("ngh", [HALF, 96], DT.float32, isOutput=False)
    wn_i = nc.declare_dram_parameter("wn", [HALF, K], DT.float32, isOutput=False)
    dd_i = nc.declare_dram_parameter("dd", [HALF, K], DT.float32, isOutput=False)
    # cst layout: [0:8]=-2cos(a), [8:16]=-2sin(a), [16:21]=r, [21:26]=r^2,
    #             [26:66]=tx, [66:106]=ty   (replicated over partitions)
    cst_i = nc.declare_dram_parameter("cst", [128, 106], DT.float32, isOutput=False)
    m3_o = nc.declare_dram_parameter("m3o", [HALF, NCELL, 3], DT.float32, isOutput=True)
    pxy_o = nc.declare_dram_parameter("pxy", [HALF, 2, K], DT.float32, isOutput=True)

    with tile.TileContext(nc) as tc, ExitStack() as ctx:
        cp = ctx.enter_context(tc.tile_pool(name="const", bufs=1))
        sp = ctx.enter_context(tc.tile_pool(name="scratch", bufs=2))
        bp = ctx.enter_context(tc.tile_pool(name="bc", bufs=2))

        NGH = cp.tile([128, NT, 96], DT.float32)
        WN = cp.tile([128, NT, K], DT.float32)
        DD = cp.tile([128, NT, K], DT.float32)
        CST = cp.tile([128, 106], DT.float32)
        nc.sync.dma_start(NGH[:], ngh_i[:].rearrange("(t p) c -> p t c", p=128))
        nc.sync.dma_start(WN[:], wn_i[:].rearrange("(t p) c -> p t c", p=128))
        nc.sync.dma_start(DD[:], dd_i[:].rearrange("(t p) c -> p t c", p=128))
        nc.sync.dma_start(CST[:], cst_i[:])
        TX = CST[:, 26:66]
        TY = CST[:, 66:106]

        KIOTA = cp.tile([128, NCELL, K], DT.int32)
        nc.gpsimd.iota(KIOTA[:], pattern=[[0, NCELL], [1, K]], base=-2147483648,
                       channel_multiplier=0)
        M32 = cp.tile([128, 1], DT.int32)
        nc.vector.memset(M32[:], -32)

        _tagn = [0]

        def nt_tile(pool=cp):
            _tagn[0] += 1
            return pool.tile([128, NT], DT.float32, tag=f"nt{_tagn[0]}",
                             name=f"nt{_tagn[0]}")

        NGH4 = NGH[:].rearrange("p t (c k) -> p t c k", c=3)

        # ---- covariance accumulation (batched over tiles) ----
        NW = cp.tile([128, NT, 96], DT.float32)
        NW4 = NW[:].rearrange("p t (c k) -> p t c k", c=3)
        nc.vector.tensor_tensor(
            out=NW4, in0=NGH4,
            in1=WN[:].rearrange("p t k -> p t () k").to_broadcast([128, NT, 3, K]),
            op=ALU.mult)
        CXX, CXY, CXZ, CYY, CYZ, CZZ = [nt_tile() for _ in range(6)]
        cov_dsts = {"xx": CXX, "xy": CXY, "xz": CXZ, "yy": CYY, "yz": CYZ, "zz": CZZ}
        pairs = [("xx", 0, 0), ("xy", 0, 1), ("xz", 0, 2),
                 ("yy", 1, 1), ("yz", 1, 2), ("zz", 2, 2)]
        for nmq, a, b in pairs:
            cj = sp.tile([128, NT, K], DT.float32, tag="covjunk")
            nc.gpsimd.tensor_tensor(out=cj[:], in0=NGH4[:, :, a, :],
                                    in1=NW4[:, :, b, :], op=ALU.mult)
            nc.vector.tensor_reduce(out=cov_dsts[nmq][:], in_=cj[:],
                                    axis=mybir.AxisListType.X, op=ALU.add)

        # ---- eigensolver on (128, NT) ----
        def tt(dst, a, bb, op):
            nc.vector.tensor_tensor(out=dst[:], in0=a[:], in1=bb[:], op=op)

        def sq_act(dst, a):
            nc.vector.tensor_tensor(out=dst[:], in0=a[:], in1=a[:], op=ALU.mult)

        Q = nt_tile()
        tt(Q, CXX, CYY, ALU.add)
        tt(Q, Q, CZZ, ALU.add)
        nc.vector.tensor_scalar_mul(Q[:], Q[:], 1.0 / 3.0)
        BXX, BYY, BZZ = nt_tile(), nt_tile(), nt_tile()
        tt(BXX, CXX, Q, ALU.subtract)
        tt(BYY, CYY, Q, ALU.subtract)
        tt(BZZ, CZZ, Q, ALU.subtract)
        P2 = nt_tile()
        T1 = nt_tile(sp)
        sq_act(P2, BXX)
        sq_act(T1, BYY)
        tt(P2, P2, T1, ALU.add)
        sq_act(T1, BZZ)
        tt(P2, P2, T1, ALU.add)
        T2 = nt_tile(sp)
        sq_act(T1, CXY)
        sq_act(T2, CXZ)
        tt(T1, T1, T2, ALU.add)
        sq_act(T2, CYZ)
        tt(T1, T1, T2, ALU.add)
        nc.vector.tensor_scalar_mul(T1[:], T1[:], 2.0)
        tt(P2, P2, T1, ALU.add)
        PP = nt_tile()
        PPX = nt_tile()
        nc.vector.tensor_scalar_mul(PPX[:], P2[:], 1.0 / 6.0)

        def polished_sqrt(dst, x, tmp):
            # ACT Sqrt is ~7e-6; one Newton step s' = (s + x/s)/2 fixes it
            nc.scalar.activation(dst[:], x[:], AF.Sqrt)
            nc.vector.tensor_scalar_max(tmp[:], dst[:], 1e-30)
            nc.vector.reciprocal(tmp[:], tmp[:])
            nc.vector.tensor_tensor(out=tmp[:], in0=x[:], in1=tmp[:], op=ALU.mult)
            nc.vector.tensor_tensor(out=dst[:], in0=dst[:], in1=tmp[:], op=ALU.add)
            nc.vector.tensor_scalar_mul(dst[:], dst[:], 0.5)

        polished_sqrt(PP, PPX, T2)
        PINV = nt_tile()
        nc.vector.tensor_scalar_max(PINV[:], PP[:], 1e-20)
        nc.vector.reciprocal(PINV[:], PINV[:])
        NBXX, NBYY, NBZZ, NBXY, NBXZ, NBYZ = [nt_tile() for _ in range(6)]
        tt(NBXX, BXX, PINV, ALU.mult)
        tt(NBYY, BYY, PINV, ALU.mult)
        tt(NBZZ, BZZ, PINV, ALU.mult)
        tt(NBXY, CXY, PINV, ALU.mult)
        tt(NBXZ, CXZ, PINV, ALU.mult)
        tt(NBYZ, CYZ, PINV, ALU.mult)
        # det(B̂)
        DET = nt_tile()
        sq_act(T1, NBYZ)                     # byz^2
        tt(T2, NBYY, NBZZ, ALU.mult)
        tt(T2, T2, T1, ALU.subtract)
        tt(DET, NBXX, T2, ALU.mult)          # + bxx (byy bzz - byz^2)
        tt(T1, NBXY, NBZZ, ALU.mult)
        tt(T2, NBYZ, NBXZ, ALU.mult)
        tt(T1, T1, T2, ALU.subtract)
        tt(T1, NBXY, T1, ALU.mult)
        tt(DET, DET, T1, ALU.subtract)       # - bxy (bxy bzz - byz bxz)
        tt(T1, NBXY, NBYZ, ALU.mult)
        tt(T2, NBYY, NBXZ, ALU.mult)
        tt(T1, T1, T2, ALU.subtract)
        tt(T1, NBXZ, T1, ALU.mult)
        tt(DET, DET, T1, ALU.add)            # + bxz (bxy byz - byy bxz)
        R2 = nt_tile()                       # 2r = det  clamped to [-2, 2]
        nc.vector.tensor_scalar_min(R2[:], DET[:], 2.0)
        nc.vector.tensor_scalar_max(R2[:], R2[:], -2.0)

        def newton(beta0):
            BETA = nt_tile()
            nc.vector.memset(BETA[:], beta0)
            FV = nt_tile(sp)
            B2 = nt_tile(sp)
            for _ in range(6):
                sq_act(B2, BETA)                              # β²
                tt(FV, B2, BETA, ALU.mult)                    # β³
                nc.vector.scalar_tensor_tensor(
                    out=T1[:], in0=BETA[:], scalar=3.0, in1=FV[:],
                    op0=ALU.mult, op1=ALU.subtract)           # 3β - β³ ... careful sign
                # T1 = (β*3) - β³  => f = β³-3β-2r = -(T1) - 2r
                tt(T1, T1, R2, ALU.add)                       # T1 = 3β - β³ + 2r = -f
                nc.vector.tensor_scalar(out=B2[:], in0=B2[:], scalar1=3.0,
                                        scalar2=-3.0, op0=ALU.mult, op1=ALU.add)  # f' = 3β²-3
                nc.vector.tensor_scalar_max(B2[:], B2[:], 1e-8)
                nc.vector.reciprocal(B2[:], B2[:])
                tt(T1, T1, B2, ALU.mult)                      # -f/f'
                tt(BETA, BETA, T1, ALU.add)                   # β - f/f'
            return BETA

        BMAX = newton(2.2)
        BMIN = newton(-2.2)
        LMAX = nt_tile()
        LMIN = nt_tile()
        tt(LMAX, PP, BMAX, ALU.mult)
        tt(LMAX, LMAX, Q, ALU.add)
        tt(LMIN, PP, BMIN, ALU.mult)
        tt(LMIN, LMIN, Q, ALU.add)

        def evec(lam):
            # columns of A - lam I
            D0, D1, D2 = nt_tile(sp), nt_tile(sp), nt_tile(sp)
            tt(D0, CXX, lam, ALU.subtract)
            tt(D1, CYY, lam, ALU.subtract)
            tt(D2, CZZ, lam, ALU.subtract)
            m0 = (D0, CXY, CXZ)
            m1 = (CXY, D1, CYZ)
            m2 = (CXZ, CYZ, D2)

            def cross(u, v):
                rx, ry, rz = nt_tile(sp), nt_tile(sp), nt_tile(sp)
                tt(rx, u[1], v[2], ALU.mult)
                tt(T1, u[2], v[1], ALU.mult)
                tt(rx, rx, T1, ALU.subtract)
                tt(ry, u[2], v[0], ALU.mult)
                tt(T1, u[0], v[2], ALU.mult)
                tt(ry, ry, T1, ALU.subtract)
                tt(rz, u[0], v[1], ALU.mult)
                tt(T1, u[1], v[0], ALU.mult)
                tt(rz, rz, T1, ALU.subtract)
                return rx, ry, rz

            def norm2(c):
                n = nt_tile(sp)
                sq_act(n, c[0])
                sq_act(T1, c[1])
                tt(n, n, T1, ALU.add)
                sq_act(T1, c[2])
                tt(n, n, T1, ALU.add)
                return n

            c01 = cross(m0, m1)
            c02 = cross(m0, m2)
            c12 = cross(m1, m2)
            n01, n02, n12 = norm2(c01), norm2(c02), norm2(c12)
            G1, G2, G3 = nt_tile(sp), nt_tile(sp), nt_tile(sp)
            tt(G1, n01, n02, ALU.is_ge)
            tt(G2, n01, n12, ALU.is_ge)
            tt(G1, G1, G2, ALU.mult)                    # pick01
            tt(G3, n02, n12, ALU.is_ge)
            U = nt_tile(sp)
            nc.vector.tensor_scalar(out=U[:], in0=G1[:], scalar1=-1.0, scalar2=1.0,
                                    op0=ALU.mult, op1=ALU.add)   # 1 - pick01
            tt(G2, U, G3, ALU.mult)                     # pick02
            nc.vector.tensor_scalar(out=G3[:], in0=G3[:], scalar1=-1.0, scalar2=1.0,
                                    op0=ALU.mult, op1=ALU.add)   # 1 - g3
            tt(G3, U, G3, ALU.mult)                     # pick12
            out = []
            for ci in range(3):
                VC = nt_tile()
                tt(VC, c01[ci], G1, ALU.mult)
                tt(T1, c02[ci], G2, ALU.mult)
                tt(VC, VC, T1, ALU.add)
                tt(T1, c12[ci], G3, ALU.mult)
                tt(VC, VC, T1, ALU.add)
                out.append(VC)
            n2v = norm2(out)
            n = nt_tile(sp)
            polished_sqrt(n, n2v, T1)
            nc.vector.tensor_scalar_max(n[:], n[:], 1e-30)
            nc.vector.reciprocal(n[:], n[:])
            for VC in out:
                tt(VC, VC, n, ALU.mult)
            return out

        ZAX = evec(LMIN)
        XAX = evec(LMAX)

        # ---- disambiguation dots (batched over tiles) ----
        def batched_dot(AX, DST, engines=("vector", "gpsimd", "vector")):
            # DST = sum_c NGH[:, :, c, :] * AX[c] broadcast over K
            tmp = sp.tile([128, NT, K], DT.float32, tag="dottmp")
            axb = [AX[c][:].rearrange("p t -> p t ()").to_broadcast([128, NT, K])
                   for c in range(3)]
            nc.vector.tensor_tensor(out=DST[:], in0=NGH4[:, :, 0, :], in1=axb[0],
                                    op=ALU.mult)
            nc.gpsimd.tensor_tensor(out=tmp[:], in0=NGH4[:, :, 1, :], in1=axb[1],
                                    op=ALU.mult)
            nc.vector.tensor_tensor(out=DST[:], in0=DST[:], in1=tmp[:], op=ALU.add)
            nc.gpsimd.tensor_tensor(out=tmp[:], in0=NGH4[:, :, 2, :], in1=axb[2],
                                    op=ALU.mult)
            nc.vector.tensor_tensor(out=DST[:], in0=DST[:], in1=tmp[:], op=ALU.add)

        DOTX = cp.tile([128, NT, K], DT.float32)
        DOTZ = cp.tile([128, NT, K], DT.float32)
        batched_dot(XAX, DOTX)
        batched_dot(ZAX, DOTZ)

        SG = cp.tile([128, NT, K], DT.float32)
        FX = nt_tile()
        FZ = nt_tile()
        for DOT, F in ((DOTX, FX), (DOTZ, FZ)):
            nc.scalar.activation(SG[:], DOT[:], AF.Sign)
            nc.vector.tensor_reduce(out=F[:], in_=SG[:], axis=mybir.AxisListType.X,
                                    op=ALU.add)
            nc.scalar.activation(F[:], F[:], AF.Sign, bias=0.5, scale=1.0)
        for c in range(3):
            tt(XAX[c], XAX[c], FX, ALU.mult)
            tt(ZAX[c], ZAX[c], FZ, ALU.mult)
        nc.vector.tensor_tensor(
            out=DOTX[:], in0=DOTX[:],
            in1=FX[:].rearrange("p t -> p t ()").to_broadcast([128, NT, K]),
            op=ALU.mult)
        # y = cross(z, x)
        YAX = []
        for (i1, i2) in ((1, 2), (2, 0), (0, 1)):
            YC = nt_tile()
            tt(YC, ZAX[i1], XAX[i2], ALU.mult)
            tt(T1, ZAX[i2], XAX[i1], ALU.mult)
            tt(YC, YC, T1, ALU.subtract)
            YAX.append(YC)
        DOTY = cp.tile([128, NT, K], DT.float32)
        batched_dot(YAX, DOTY)

        # ---- projections (batched over all tiles) -> PXY (p, t, xy, k) ----
        PXY = cp.tile([128, NT, 2, K], DT.float32)
        PX = PXY[:][:, :, 0, :]
        PY = PXY[:][:, :, 1, :]
        SC = cp.tile([128, NT, K], DT.float32)
        U2 = cp.tile([128, NT, K], DT.float32)
        nc.vector.tensor_tensor(out=SC[:], in0=DOTX[:], in1=DOTX[:], op=ALU.mult)
        nc.vector.tensor_tensor(out=U2[:], in0=DOTY[:], in1=DOTY[:], op=ALU.mult)
        nc.vector.tensor_tensor(out=U2[:], in0=SC[:], in1=U2[:], op=ALU.add)
        nc.scalar.activation(SC[:], U2[:], AF.Sqrt)
        # one Newton step: s' = 0.5 (s + u/s) makes sqrt correctly-rounded-ish
        RCN = cp.tile([128, NT, K], DT.float32)
        nc.vector.tensor_scalar_max(RCN[:], SC[:], 1e-30)
        nc.vector.reciprocal(RCN[:], RCN[:])
        nc.vector.tensor_tensor(out=RCN[:], in0=U2[:], in1=RCN[:], op=ALU.mult)
        nc.vector.tensor_tensor(out=SC[:], in0=SC[:], in1=RCN[:], op=ALU.add)
        nc.vector.tensor_scalar(out=SC[:], in0=SC[:], scalar1=0.5, scalar2=EPS,
                                op0=ALU.mult, op1=ALU.add)
        nc.vector.reciprocal(SC[:], SC[:])
        nc.vector.tensor_tensor(out=SC[:], in0=SC[:], in1=DD[:], op=ALU.mult)
        nc.vector.tensor_tensor(out=PX, in0=DOTX[:], in1=SC[:], op=ALU.mult)
        nc.vector.tensor_tensor(out=PY, in0=DOTY[:], in1=SC[:], op=ALU.mult)
        # S2 = px^2 + py^2
        S2 = cp.tile([128, NT, K], DT.float32)
        S2T = cp.tile([128, NT, K], DT.float32)
        nc.vector.tensor_tensor(out=S2[:], in0=PX, in1=PX, op=ALU.mult)
        nc.vector.tensor_tensor(out=S2T[:], in0=PY, in1=PY, op=ALU.mult)
        nc.vector.tensor_tensor(out=S2[:], in0=S2[:], in1=S2T[:], op=ALU.add)
        nc.sync.dma_start(pxy_o[:].rearrange("(t p) x k -> p t x k", p=128), PXY[:])

        # ---- BC selection per tile ----
        # Key for cell (i,j), slot k:  d2 = (S2[k] + r_i^2) + r_i * W8[j,k]
        # with W8[j,k] = -2 (cos_j px[k] + sin_j py[k]); then pack slot bits
        # and take the top-3 keys per cell via MAX8. Winner coordinates are
        # gathered on the host from pxy_o (it gathers pidx anyway).
        COSB = CST[:, 0:8].rearrange("p a -> p () a ()").to_broadcast([128, NT, A, K])
        SINB = CST[:, 8:16].rearrange("p a -> p () a ()").to_broadcast([128, NT, A, K])
        RB = CST[:, 16:21].rearrange("p r -> p r () ()").to_broadcast([128, R, A, K])
        R2B = CST[:, 21:26].rearrange("p r -> p () r ()").to_broadcast([128, NT, R, K])
        # W8 and S2+r^2 batched over all tiles
        W8A = cp.tile([128, NT, A, K], DT.float32)
        T8A = cp.tile([128, NT, A, K], DT.float32)
        pxab = PX.rearrange("p t k -> p t () k").to_broadcast([128, NT, A, K])
        pyab = PY.rearrange("p t k -> p t () k").to_broadcast([128, NT, A, K])
        nc.gpsimd.tensor_tensor(out=T8A[:], in0=pxab, in1=COSB, op=ALU.mult)
        nc.gpsimd.tensor_tensor(out=W8A[:], in0=pyab, in1=SINB, op=ALU.mult)
        nc.gpsimd.tensor_tensor(out=W8A[:], in0=W8A[:], in1=T8A[:], op=ALU.add)
        S2RA = cp.tile([128, NT, R, K], DT.float32)
        nc.gpsimd.tensor_tensor(
            out=S2RA[:],
            in0=S2[:].rearrange("p t k -> p t () k").to_broadcast([128, NT, R, K]),
            in1=R2B, op=ALU.add)
        for t in range(NT):
            RW = bp.tile([128, R, A, K], DT.float32, tag="rw")
            nc.gpsimd.tensor_tensor(
                out=RW[:], in0=RB,
                in1=W8A[:, t].rearrange("p a k -> p () a k").to_broadcast(
                    [128, R, A, K]),
                op=ALU.mult)
            KEY = bp.tile([128, R, A, K], DT.float32, tag="key")
            nc.gpsimd.tensor_tensor(
                out=KEY[:], in0=RW[:],
                in1=S2RA[:, t].rearrange("p r k -> p r () k").to_broadcast(
                    [128, R, A, K]),
                op=ALU.add)
            NKEY = bp.tile([128, NCELL, K], DT.float32, tag="nkey", bufs=3)
            nc.vector.scalar_tensor_tensor(
                out=NKEY[:].bitcast(DT.int32),
                in0=KEY[:].rearrange("p r a k -> p (r a) k").bitcast(DT.int32),
                scalar=M32[:], in1=KIOTA[:], op0=ALU.bitwise_and,
                op1=ALU.bitwise_or)
            M8 = bp.tile([128, NCELL, 8], DT.float32, tag="m8", bufs=3)
            for ra in range(NCELL):
                nc.vector.max(out=M8[:, ra, :], in_=NKEY[:, ra, :])
            M3C = bp.tile([128, NCELL, 3], DT.float32, tag="m3c", bufs=3)
            nc.scalar.copy(M3C[:], M8[:, :, 0:3])
            nc.sync.dma_start(m3_o[t * 128:(t + 1) * 128, :, :], M3C[:])

    split_sync_waits(nc)
    return nc


# ---------------------------------------------------------------------------
# Host glue
# ---------------------------------------------------------------------------


def _split16(x):
    """f32 -> (hi, lo) fp16 pair with hi + lo ~= x."""
    hi = x.astype(np.float16)
    lo = (x - hi.astype(f32)).astype(np.float16)
    return hi, lo


def host_prep_phase1(vertices):
    """vertices (4, 4096, 3) -> list of 8 input maps (13-row fp16 split)."""
    f16 = np.float16
    maps = []
    for core in range(8):
        b, h = core // 2, core % 2
        verts = np.ascontiguousarray(vertices[b], dtype=f32)
        sq = (verts.astype(np.float64) ** 2).sum(-1).astype(f32)
        bh, bl = _split16(verts.T)                     # (3, V) each
        sph, spl = _split16(-sq[None, :])              # (1, V)
        onev = np.ones((1, V), f16)
        pt5 = np.concatenate(
            [bh, bh, bl, bl, sph, spl, onev, onev], axis=0).astype(f16)
        Q = verts[h * HALF:(h + 1) * HALF]
        qsq = sq[h * HALF:(h + 1) * HALF]
        ah, al = _split16(2.0 * Q.T.astype(f32))       # (3, HALF)
        sqh, sql = _split16(-qsq[None, :])
        oneq = np.ones((1, HALF), f16)
        qt5 = np.concatenate(
            [ah, al, ah, al, oneq, oneq, sqh, sql], axis=0).astype(f16)
        maps.append({"pt5": np.ascontiguousarray(pt5),
                     "qt5": np.ascontiguousarray(qt5)})
    return maps


def host_merge(candv, verts, Q):
    """Decode packed candidates, refine exactly, take top-33 by (d2, index).

    candv (HALF, CAND) f32: bits = (-d2 & ~127) | chunk_local_idx, column c
    belongs to chunk c // 8. The device d2 is approximate (fp16-split matmul
    + 7 masked mantissa bits); the top ~48 candidates are re-scored with
    exact f32 distances so the top-33 boundary is noise-free.
    -> nbr (HALF,32) int64, d (HALF,32), radius (HALF,).
    """
    NCAND = 48
    bits = candv.view(np.uint32)
    j = (bits & np.uint32(CHUNKW - 1)).astype(np.int64)
    d2m = -(bits & np.uint32((0xFFFFFFFF << 8) & 0xFFFFFFFF)).view(f32)
    chunk = np.arange(CAND, dtype=np.int64) // 8
    gidx = chunk[None, :] * CHUNKW + j
    part = np.argpartition(d2m, NCAND, axis=1)[:, :NCAND]
    cd = np.take_along_axis(gidx, part, axis=1)              # (HALF, 48)
    diff = verts[cd] - Q[:, None, :]
    d2x = np.einsum("qkc,qkc->qk", diff, diff, dtype=f32).astype(f32)
    order = np.lexsort((cd, d2x), axis=1)[:, :33]
    vals = np.take_along_axis(d2x, order, axis=1)
    idxs = np.take_along_axis(cd, order, axis=1)
    d33 = np.sqrt(np.maximum(vals, 0.0)).astype(f32)
    return idxs[:, :32], d33[:, :32], d33[:, 32]


def host_prep_phase2(vertices, template, p1_results):
    """Build phase-2 input maps + per-core nbr tables from phase-1 outputs."""
    template = np.asarray(template, f32)
    tx = template[..., 0].reshape(-1).astype(f32)
    ty = template[..., 1].reshape(-1).astype(f32)
    # polar factorization of the template grid (it is a polar r x a grid)
    r64 = np.hypot(template[..., 0].astype(np.float64),
                   template[..., 1].astype(np.float64)).mean(axis=1)  # (R,)
    ang = np.arctan2(template[-1, :, 1].astype(np.float64),
                     template[-1, :, 0].astype(np.float64))           # (A,)
    cst_row = np.concatenate([
        -2.0 * np.cos(ang), -2.0 * np.sin(ang), r64, r64 * r64,
        tx.astype(np.float64), ty.astype(np.float64)]).astype(f32)
    cst = np.ascontiguousarray(np.broadcast_to(cst_row[None, :], (128, 106)))
    maps, nbrs = [], []
    for core in range(8):
        b, h = core // 2, core % 2
        verts = np.ascontiguousarray(vertices[b], dtype=f32)
        cv = p1_results[core]["candv"]
        Q = verts[h * HALF:(h + 1) * HALF]
        nbr, d, radius = host_merge(cv, verts, Q)
        neigh = (verts[nbr] - Q[:, None, :]).astype(f32)          # (HALF, 32, 3)
        ngh = np.ascontiguousarray(neigh.transpose(0, 2, 1).reshape(HALF, 96))
        w = (radius[:, None] - d).astype(f32)
        wn = (w / (w.sum(1, keepdims=True, dtype=f32) + f32(EPS))).astype(f32)
        maps.append({"ngh": ngh, "wn": np.ascontiguousarray(wn),
                     "dd": np.ascontiguousarray(d), "cst": cst})
        nbrs.append(nbr)
    return maps, nbrs


def host_assemble(p2_results, nbrs, template):
    """Decode slots, gather winner coords, barycentric weights, assemble output."""
    template = np.asarray(template, f32)
    txy = template.reshape(NCELL, 2)
    out = np.zeros((B, V, R, A, 3, 2), f32)
    one = f32(1.0)
    for core in range(8):
        b, h = core // 2, core % 2
        m3 = np.ascontiguousarray(p2_results[core]["m3o"])        # (HALF, 40, 3)
        pxy = p2_results[core]["pxy"]                             # (HALF, 2, 32)
        k3 = (m3.view(np.int32) & 31).astype(np.int64)            # (HALF, 40, 3)
        nbr = nbrs[core]                                          # (HALF, 32)
        pidx = np.take_along_axis(nbr[:, None, :].repeat(NCELL, 1), k3, axis=2)
        k3f = k3.reshape(HALF, NCELL * 3)
        px = np.take_along_axis(pxy[:, 0, :], k3f, axis=1).reshape(HALF, NCELL, 3)
        py = np.take_along_axis(pxy[:, 1, :], k3f, axis=1).reshape(HALF, NCELL, 3)
        p0x, p1x, p2x = px[..., 0], px[..., 1], px[..., 2]
        p0y, p1y, p2y = py[..., 0], py[..., 1], py[..., 2]
        v0x, v0y = p2x - p0x, p2y - p0y
        v1x, v1y = p1x - p0x, p1y - p0y
        v2x = txy[None, :, 0] - p0x
        v2y = txy[None, :, 1] - p0y
        d00 = v0x * v0x + v0y * v0y
        d01 = v0x * v1x + v0y * v1y
        d02 = v0x * v2x + v0y * v2y
        d11 = v1x * v1x + v1y * v1y
        d12 = v1x * v2x + v1y * v2y
        den = d00 * d11 - d01 * d01 + f32(1e-6)
        w2 = (d11 * d02 - d01 * d12) / den
        w1 = (d00 * d12 - d01 * d02) / den
        w0 = one - w2 - w1
        weights = np.stack([w2, w1, w0], axis=-1).astype(f32)     # (HALF, 40, 3)
        sl = slice(h * HALF, (h + 1) * HALF)
        out[b, sl, ..., 0] = pidx.reshape(HALF, R, A, 3).astype(f32)
        out[b, sl, ..., 1] = weights.reshape(HALF, R, A, 3)
    return out


_PROGS = {}


def _prog(name):
    if name not in _PROGS:
        _PROGS[name] = build_phase1() if name == "p1" else build_phase2()
    return _PROGS[name]


def run_phase1(vertices, trace=False):
    maps = host_prep_phase1(vertices)
    return run_bass_kernel_spmd(_prog("p1"), maps, list(range(8)), trace=trace)


def kernel(vertices, template, trace=False, _timing=None):
    vertices = np.asarray(vertices, f32)
    template = np.asarray(template, f32)
    r1 = run_bass_kernel_spmd(_prog("p1"), host_prep_phase1(vertices),
                              list(range(8)), trace=trace)
    maps2, nbrs = host_prep_phase2(vertices, template, r1.results)
    r2 = run_bass_kernel_spmd(_prog("p2"), maps2, list(range(8)), trace=trace)
    if _timing is not None:
        _timing["phase1"] = r1
        _timing["phase2"] = r2
        _timing["maps2"] = maps2
        _timing["nbrs"] = nbrs
    return host_assemble(r2.results, nbrs, template)


if __name__ == "__main__":
    # Phase-1 standalone check against exact numpy KNN.
    cache = np.load("/root/problem/dev_cache/ref.npz")
    vertices = cache["vertices"]
    res = run_phase1(vertices)
    nbad = 0
    for core in range(8):
        b, h = core // 2, core % 2
        verts = vertices[b].astype(f32)
        Q = verts[h * HALF:(h + 1) * HALF]
        d2 = ((Q[:, None, :] - verts[None, :, :]) ** 2).sum(-1)
        ref_order = np.argsort(d2, axis=1, kind="stable")[:, :33]
        nbr, d, rad = host_merge(res.results[core]["candv"], verts, Q)
        rnbr = ref_order[:, :32]
        idx_match = (np.sort(nbr, 1) == np.sort(rnbr, 1)).mean()
        rrad = np.sqrt(np.take_along_axis(d2, ref_order[:, 32:33], axis=1)[:, 0])
        print(f"core {core}: top32 set match={idx_match:.6f} "
              f"rad maxdiff={np.abs(rad-rrad).max():.2e}")
        nbad += (np.sort(nbr, 1) != np.sort(rnbr, 1)).sum()
    print("total nbr mismatches vs exact:", nbad)

